# revision 21
# baseline (speedup 1.0000x reference)
"""Segment-prefix max kernel for Trainium2 (8 NeuronCores, SPMD).

Problem: x [1048576, 128] f32, 2048 uniform segments of 512 rows each;
out[i, :] = max over the first (512 - window_size + 1) rows of segment i.

Strategy (memory-bound, streams ~512 MiB from HBM at the device wall):
  - Shard segments across 8 cores: core c gets rows [c*131072, (c+1)*131072)
    and produces out rows [c*256, (c+1)*256). No cross-core communication.
  - Per core, 16 tiles of 4 MiB (16 segments): partition p holds runs p and
    128+p of the tile's 256 consecutive 32-row runs — one 16 KiB contiguous
    DMA run per partition per fill (vs 2 KiB naive). One big DMA per tile,
    alternating between the sync and scalar HWDGE rings (each ring is a
    serial ~200 GB/s pipe, so transfers must be large and few).
  - Run 15 of each segment ends with the invalid window-tail rows
    (row >= count). A tiny strided DMA on the otherwise-idle gpsimd (SWDGE)
    ring overwrites those slots with duplicates of valid rows (duplicates
    are harmless for max), so the reduce can use all 16 runs untouched.
    Total HBM traffic is within 0.4% of the theoretical minimum.
  - The 32 -> 1 fold along the free axis runs on DVE as a binary tree; the
    first level reads f32 and writes bf16, middle levels run in bf16 at 2x
    DVE throughput, the last level emits f32 (rel tolerance 2e-2 >> bf16's
    ~4e-3 rounding).
  - Cross-partition max (each segment = 16 consecutive partitions of one
    fill) goes through a PE transpose (identity matmul into PSUM) and one
    DVE reduce_max over each segment's 16 columns into a [128 d, n_seg]
    column accumulator.
  - Final columns are PE-transposed back to row-major [n_seg, 128] chunks
    and stored with a single DMA.
"""

import numpy as np

import concourse.bacc as bacc
import concourse.bass as bass
import concourse.tile as tile
from concourse import mybir
from concourse.bass_utils import run_bass_kernel_spmd
from concourse.masks import make_identity

N_CORES = 8
SEG_LEN = 512
D = 128
J = 32  # rows per run (16 KiB contiguous DMA run)
RUNS = SEG_LEN // J  # 16 runs per segment
FILLS = 2  # fills per tile; tile = FILLS * 2 MiB
SEGS_PER_FILL = 128 // RUNS  # 8
SEGS_PER_TILE = FILLS * SEGS_PER_FILL  # 16 segments, 4 MiB tiles

_PROGRAM_CACHE: dict = {}


def _build_program(n_seg_core: int, count: int) -> bacc.Bacc:
    """Bass program for one core: n_seg_core segments, max over first
    `count` rows of each. Requires SEG_LEN - J < count <= SEG_LEN and
    count >= 2*J (so duplicate source rows exist below the cut)."""
    assert SEG_LEN - J < count <= SEG_LEN
    assert count >= 2 * J
    rows = n_seg_core * SEG_LEN
    n_tiles = n_seg_core // SEGS_PER_TILE
    n_bad = SEG_LEN - count  # invalid trailing rows per segment
    has_tail = n_bad > 0
    f32 = mybir.dt.float32
    bf16 = mybir.dt.bfloat16

    nc = bacc.Bacc("TRN2", target_bir_lowering=False, debug=False)
    x_in = nc.dram_tensor("x", [rows, D], f32, kind="ExternalInput")
    out_t = nc.dram_tensor("out", [n_seg_core, D], f32, kind="ExternalOutput")

    # tile t, partition p, fill f -> run 256*t + 128*f + p
    x_tile = x_in.rearrange("(t f p j) d -> t p f j d", f=FILLS, p=128, j=J)
    # patch source: tile t, seg-in-fill l, fill f, last valid rows
    x_rows = x_in.rearrange(
        "(t f l q) d -> t l f q d", f=FILLS, l=SEGS_PER_FILL, q=SEG_LEN
    )

    rings = [nc.sync, nc.scalar]

    with tile.TileContext(nc) as tc:
        with (
            tc.tile_pool(name="io", bufs=4) as io_pool,
            tc.tile_pool(name="work", bufs=3) as work_pool,
            tc.tile_pool(name="psum", bufs=4, space="PSUM") as psum_pool,
            tc.tile_pool(name="psum2", bufs=2, space="PSUM") as psum_pool2,
            tc.tile_pool(name="consts", bufs=1) as consts,
        ):
            ident_f = consts.tile([128, 128], f32)
            make_identity(nc, ident_f)
            outbuf = consts.tile([128, n_seg_core], f32)

            for t in range(n_tiles):
                tl = io_pool.tile([128, FILLS, J, D], f32, tag="tl")
                g0 = t * SEGS_PER_TILE
                rings[t % 2].dma_start(out=tl, in_=x_tile[t])
                if has_tail:
                    # overwrite the invalid trailing rows of each segment's
                    # last run with duplicates of the last valid rows
                    nc.gpsimd.dma_start(
                        out=tl[RUNS - 1 :: RUNS, :, J - n_bad :, :],
                        in_=x_rows[t, :, :, count - n_bad : count],
                    )

                # fold 32 -> 1 along j: f32 -> bf16, bf16 tree, bf16 -> f32
                w = work_pool.tile([128, FILLS, J // 2, D], bf16, tag="w")
                nc.vector.tensor_max(
                    out=w, in0=tl[:, :, : J // 2], in1=tl[:, :, J // 2 :]
                )
                k = J // 2
                while k > 2:
                    k //= 2
                    nc.vector.tensor_max(
                        out=w[:, :, :k], in0=w[:, :, :k], in1=w[:, :, k : 2 * k]
                    )
                wf = work_pool.tile([128, FILLS, D], f32, tag="wf")
                nc.vector.tensor_max(out=wf, in0=w[:, :, 0], in1=w[:, :, 1])

                # per fill: transpose, reduce each segment's 16 columns
                for f in range(FILLS):
                    gf = g0 + f * SEGS_PER_FILL
                    pt = psum_pool.tile([128, SEGS_PER_FILL, RUNS], f32, tag="pt")
                    nc.tensor.transpose(
                        pt.rearrange("p a b -> p (a b)"), wf[:, f], ident_f
                    )
                    nc.vector.reduce_max(
                        out=outbuf[:, gf : gf + SEGS_PER_FILL],
                        in_=pt,
                        axis=mybir.AxisListType.X,
                    )

            # outbuf is [128 d, n_seg_core]; transpose back to [seg, d]
            ot = io_pool.tile([128, n_seg_core // 128, 128], f32, tag="ot")
            for c in range(n_seg_core // 128):
                pt = psum_pool2.tile([128, 128], f32, tag="ot_ps")
                nc.tensor.transpose(pt, outbuf[:, c * 128 : (c + 1) * 128], ident_f)
                nc.scalar.copy(ot[:, c], pt)
            nc.sync.dma_start(
                out=out_t.rearrange("(c p) d -> p c d", p=128), in_=ot
            )
    nc.compile()
    return nc


def _numpy_fallback(x: np.ndarray, sizes: np.ndarray, w: int) -> np.ndarray:
    ends = np.cumsum(sizes)
    starts = ends - sizes
    out = np.full((sizes.shape[0], x.shape[1]), -np.inf, dtype=np.float32)
    for i in range(sizes.shape[0]):
        c = int(sizes[i]) - w + 1
        if c > 0:
            out[i] = x[int(starts[i]) : int(starts[i]) + c].max(axis=0)
    return out


def kernel(x, sizes, window_size) -> np.ndarray:
    x = np.ascontiguousarray(np.asarray(x, dtype=np.float32))
    sizes = np.asarray(sizes)
    w = int(np.asarray(window_size))
    n_seg = sizes.shape[0]
    count = SEG_LEN - w + 1

    uniform = (
        x.ndim == 2
        and x.shape[1] == D
        and bool((sizes == SEG_LEN).all())
        and x.shape[0] == n_seg * SEG_LEN
        and n_seg % (N_CORES * SEGS_PER_TILE) == 0
        and (n_seg // N_CORES) % 128 == 0
        and SEG_LEN - J < count <= SEG_LEN
        and count >= 2 * J
    )
    if not uniform:
        return _numpy_fallback(x, sizes, w)

    n_seg_core = n_seg // N_CORES
    key = (n_seg_core, count)
    if key not in _PROGRAM_CACHE:
        _PROGRAM_CACHE[key] = _build_program(n_seg_core, count)
    nc = _PROGRAM_CACHE[key]

    shards = np.split(x, N_CORES, axis=0)
    in_maps = [{"x": s} for s in shards]
    res = run_bass_kernel_spmd(nc, in_maps, core_ids=list(range(N_CORES)))
    return np.concatenate([r["out"] for r in res.results], axis=0)


# revision 22
# speedup vs baseline: 1.0439x; 1.0439x over previous
"""Segment-prefix max kernel for Trainium2 (8 NeuronCores, SPMD).

Problem: x [1048576, 128] f32, 2048 uniform segments of 512 rows each;
out[i, :] = max over the first (512 - window_size + 1) rows of segment i.

Strategy (memory-bound, streams ~512 MiB from HBM at the device wall):
  - Shard segments across 8 cores: core c gets rows [c*131072, (c+1)*131072)
    and produces out rows [c*256, (c+1)*256). No cross-core communication.
  - Per core, 16 tiles of 4 MiB (16 segments): partition p holds runs p and
    128+p of the tile's 256 consecutive 32-row runs — one 16 KiB contiguous
    DMA run per partition per fill (vs 2 KiB naive). One big DMA per tile,
    alternating between the sync and scalar HWDGE rings (each ring is a
    serial ~200 GB/s pipe, so transfers must be large and few).
  - Run 15 of each segment ends with the invalid window-tail rows
    (row >= count). A tiny strided DMA on the otherwise-idle gpsimd (SWDGE)
    ring overwrites those slots with duplicates of valid rows (duplicates
    are harmless for max), so the reduce can use all 16 runs untouched.
    Total HBM traffic is within 0.4% of the theoretical minimum.
  - The 32 -> 1 fold along the free axis runs on DVE as a binary tree; the
    first level reads f32 and writes bf16, middle levels run in bf16 at 2x
    DVE throughput, the last level emits f32 (rel tolerance 2e-2 >> bf16's
    ~4e-3 rounding).
  - Cross-partition max (each segment = 16 consecutive partitions of one
    fill) goes through a PE transpose (identity matmul into PSUM) and one
    DVE reduce_max over each segment's 16 columns into a [128 d, n_seg]
    column accumulator.
  - Final columns are PE-transposed back to row-major [n_seg, 128] chunks
    and stored with a single DMA.
"""

import numpy as np

import concourse.bacc as bacc
import concourse.bass as bass
import concourse.tile as tile
from concourse import mybir
from concourse.bass_utils import run_bass_kernel_spmd
from concourse.masks import make_identity

N_CORES = 8
SEG_LEN = 512
D = 128
J = 32  # rows per run (16 KiB contiguous DMA run)
RUNS = SEG_LEN // J  # 16 runs per segment
FILLS = 2  # fills per tile; tile = FILLS * 2 MiB
SEGS_PER_FILL = 128 // RUNS  # 8
SEGS_PER_TILE = FILLS * SEGS_PER_FILL  # 16 segments, 4 MiB tiles

_PROGRAM_CACHE: dict = {}


def _build_program(n_seg_core: int, count: int) -> bacc.Bacc:
    """Bass program for one core: n_seg_core segments, max over first
    `count` rows of each. Requires SEG_LEN - J < count <= SEG_LEN and
    count >= 2*J (so duplicate source rows exist below the cut)."""
    assert SEG_LEN - J < count <= SEG_LEN
    assert count >= 2 * J
    rows = n_seg_core * SEG_LEN
    n_tiles = n_seg_core // SEGS_PER_TILE
    n_bad = SEG_LEN - count  # invalid trailing rows per segment
    has_tail = n_bad > 0
    f32 = mybir.dt.float32
    bf16 = mybir.dt.bfloat16

    nc = bacc.Bacc("TRN2", target_bir_lowering=False, debug=False)
    x_in = nc.dram_tensor("x", [rows, D], f32, kind="ExternalInput")
    out_t = nc.dram_tensor("out", [n_seg_core, D], f32, kind="ExternalOutput")

    # tile t, partition p, fill f -> run 256*t + 128*f + p
    x_tile = x_in.rearrange("(t f p j) d -> t p f j d", f=FILLS, p=128, j=J)
    # patch source: tile t, seg-in-fill l, fill f, last valid rows
    x_rows = x_in.rearrange(
        "(t f l q) d -> t l f q d", f=FILLS, l=SEGS_PER_FILL, q=SEG_LEN
    )

    rings = [nc.sync, nc.scalar]

    with tile.TileContext(nc) as tc:
        with (
            tc.tile_pool(name="io", bufs=5) as io_pool,
            tc.tile_pool(name="work", bufs=3) as work_pool,
            tc.tile_pool(name="psum", bufs=4, space="PSUM") as psum_pool,
            tc.tile_pool(name="psum2", bufs=2, space="PSUM") as psum_pool2,
            tc.tile_pool(name="consts", bufs=1) as consts,
        ):
            ident_f = consts.tile([128, 128], f32)
            make_identity(nc, ident_f)
            outbuf = consts.tile([128, n_seg_core], f32)
            ot = consts.tile([128, n_seg_core // 128, 128], f32)

            for t in range(n_tiles):
                tl = io_pool.tile([128, FILLS, J, D], f32, tag="tl")
                g0 = t * SEGS_PER_TILE
                rings[t % 2].dma_start(out=tl, in_=x_tile[t])
                if has_tail:
                    # overwrite the invalid trailing rows of each segment's
                    # last run with duplicates of the last valid rows
                    patch_ring = rings[t % 2] if t == n_tiles - 1 else nc.gpsimd
                    patch_ring.dma_start(
                        out=tl[RUNS - 1 :: RUNS, :, J - n_bad :, :],
                        in_=x_rows[t, :, :, count - n_bad : count],
                    )

                # fold 32 -> 1 along j: f32 -> bf16, bf16 tree, bf16 -> f32
                w = work_pool.tile([128, FILLS, J // 2, D], bf16, tag="w")
                nc.vector.tensor_max(
                    out=w, in0=tl[:, :, : J // 2], in1=tl[:, :, J // 2 :]
                )
                k = J // 2
                while k > 2:
                    k //= 2
                    nc.vector.tensor_max(
                        out=w[:, :, :k], in0=w[:, :, :k], in1=w[:, :, k : 2 * k]
                    )
                wf = work_pool.tile([128, FILLS, D], f32, tag="wf")
                nc.vector.tensor_max(out=wf, in0=w[:, :, 0], in1=w[:, :, 1])

                # per fill: transpose, reduce each segment's 16 columns
                for f in range(FILLS):
                    gf = g0 + f * SEGS_PER_FILL
                    pt = psum_pool.tile([128, SEGS_PER_FILL, RUNS], f32, tag="pt")
                    nc.tensor.transpose(
                        pt.rearrange("p a b -> p (a b)"), wf[:, f], ident_f
                    )
                    nc.vector.reduce_max(
                        out=outbuf[:, gf : gf + SEGS_PER_FILL],
                        in_=pt,
                        axis=mybir.AxisListType.X,
                    )

                # outbuf chunk c = segs [c*128, (c+1)*128) is complete once
                # tile (c+1)*8 - 1 has reduced; transpose it back to
                # row-major [seg, d] immediately instead of at the end
                if (t + 1) * SEGS_PER_TILE % 128 == 0 and f == FILLS - 1:
                    c = ((t + 1) * SEGS_PER_TILE) // 128 - 1
                    ptc = psum_pool2.tile([128, 128], f32, tag="ot_ps")
                    nc.tensor.transpose(
                        ptc, outbuf[:, c * 128 : (c + 1) * 128], ident_f
                    )
                    nc.scalar.copy(ot[:, c], ptc)

            nc.sync.dma_start(
                out=out_t.rearrange("(c p) d -> p c d", p=128), in_=ot
            )
    nc.compile()
    return nc


def _numpy_fallback(x: np.ndarray, sizes: np.ndarray, w: int) -> np.ndarray:
    ends = np.cumsum(sizes)
    starts = ends - sizes
    out = np.full((sizes.shape[0], x.shape[1]), -np.inf, dtype=np.float32)
    for i in range(sizes.shape[0]):
        c = int(sizes[i]) - w + 1
        if c > 0:
            out[i] = x[int(starts[i]) : int(starts[i]) + c].max(axis=0)
    return out


def kernel(x, sizes, window_size) -> np.ndarray:
    x = np.ascontiguousarray(np.asarray(x, dtype=np.float32))
    sizes = np.asarray(sizes)
    w = int(np.asarray(window_size))
    n_seg = sizes.shape[0]
    count = SEG_LEN - w + 1

    uniform = (
        x.ndim == 2
        and x.shape[1] == D
        and bool((sizes == SEG_LEN).all())
        and x.shape[0] == n_seg * SEG_LEN
        and n_seg % (N_CORES * SEGS_PER_TILE) == 0
        and (n_seg // N_CORES) % 128 == 0
        and SEG_LEN - J < count <= SEG_LEN
        and count >= 2 * J
    )
    if not uniform:
        return _numpy_fallback(x, sizes, w)

    n_seg_core = n_seg // N_CORES
    key = (n_seg_core, count)
    if key not in _PROGRAM_CACHE:
        _PROGRAM_CACHE[key] = _build_program(n_seg_core, count)
    nc = _PROGRAM_CACHE[key]

    shards = np.split(x, N_CORES, axis=0)
    in_maps = [{"x": s} for s in shards]
    res = run_bass_kernel_spmd(nc, in_maps, core_ids=list(range(N_CORES)))
    return np.concatenate([r["out"] for r in res.results], axis=0)


# revision 23
# speedup vs baseline: 1.0626x; 1.0179x over previous
"""Segment-prefix max kernel for Trainium2 (8 NeuronCores, SPMD).

Problem: x [1048576, 128] f32, 2048 uniform segments of 512 rows each;
out[i, :] = max over the first (512 - window_size + 1) rows of segment i.

Strategy (memory-bound, streams ~512 MiB from HBM at the device wall):
  - Shard segments across 8 cores: core c gets rows [c*131072, (c+1)*131072)
    and produces out rows [c*256, (c+1)*256). No cross-core communication.
  - Per core, 32 tiles of 2 MiB (8 segments): partition p holds run p of
    the tile's 128 consecutive 32-row runs — one 16 KiB contiguous DMA run
    per partition, sequential in DRAM. One big DMA per tile,
    alternating between the sync and scalar HWDGE rings (each ring is a
    serial ~200 GB/s pipe, so transfers must be large and few).
  - Run 15 of each segment ends with the invalid window-tail rows
    (row >= count). A tiny strided DMA on the otherwise-idle gpsimd (SWDGE)
    ring overwrites those slots with duplicates of valid rows (duplicates
    are harmless for max), so the reduce can use all 16 runs untouched.
    Total HBM traffic is within 0.4% of the theoretical minimum.
  - The 32 -> 1 fold along the free axis runs on DVE as a binary tree; the
    first level reads f32 and writes bf16, middle levels run in bf16 at 2x
    DVE throughput, the last level emits f32 (rel tolerance 2e-2 >> bf16's
    ~4e-3 rounding).
  - Cross-partition max (each segment = 16 consecutive partitions of one
    fill) goes through a PE transpose (identity matmul into PSUM) and one
    DVE reduce_max over each segment's 16 columns into a [128 d, n_seg]
    column accumulator.
  - Final columns are PE-transposed back to row-major [n_seg, 128] chunks
    and stored with a single DMA.
"""

import numpy as np

import concourse.bacc as bacc
import concourse.bass as bass
import concourse.tile as tile
from concourse import mybir
from concourse.bass_utils import run_bass_kernel_spmd
from concourse.masks import make_identity

N_CORES = 8
SEG_LEN = 512
D = 128
J = 32  # rows per run (16 KiB contiguous DMA run)
RUNS = SEG_LEN // J  # 16 runs per segment
SEGS_PER_TILE = 128 // RUNS  # 8 segments, 2 MiB tiles

_PROGRAM_CACHE: dict = {}


def _build_program(n_seg_core: int, count: int) -> bacc.Bacc:
    """Bass program for one core: n_seg_core segments, max over first
    `count` rows of each. Requires SEG_LEN - J < count <= SEG_LEN and
    count >= 2*J (so duplicate source rows exist below the cut)."""
    assert SEG_LEN - J < count <= SEG_LEN
    assert count >= 2 * J
    rows = n_seg_core * SEG_LEN
    n_tiles = n_seg_core // SEGS_PER_TILE
    n_bad = SEG_LEN - count  # invalid trailing rows per segment
    has_tail = n_bad > 0
    f32 = mybir.dt.float32
    bf16 = mybir.dt.bfloat16

    nc = bacc.Bacc("TRN2", target_bir_lowering=False, debug=False)
    x_in = nc.dram_tensor("x", [rows, D], f32, kind="ExternalInput")
    out_t = nc.dram_tensor("out", [n_seg_core, D], f32, kind="ExternalOutput")

    # tile t, partition p -> run 128*t + p (transfers are sequential in DRAM)
    x_tile = x_in.rearrange("(t p j) d -> t p j d", p=128, j=J)
    # patch source: tile t, seg-in-tile l, last valid rows
    x_rows = x_in.rearrange("(t l q) d -> t l q d", l=SEGS_PER_TILE, q=SEG_LEN)

    rings = [nc.sync, nc.scalar]

    with tile.TileContext(nc) as tc:
        with (
            tc.tile_pool(name="io", bufs=9) as io_pool,
            tc.tile_pool(name="work", bufs=3) as work_pool,
            tc.tile_pool(name="psum", bufs=4, space="PSUM") as psum_pool,
            tc.tile_pool(name="psum2", bufs=2, space="PSUM") as psum_pool2,
            tc.tile_pool(name="consts", bufs=1) as consts,
        ):
            ident_f = consts.tile([128, 128], f32)
            make_identity(nc, ident_f)
            outbuf = consts.tile([128, n_seg_core], f32)
            ot = consts.tile([128, n_seg_core // 128, 128], f32)

            for t in range(n_tiles):
                tl = io_pool.tile([128, J, D], f32, tag="tl")
                g0 = t * SEGS_PER_TILE
                rings[t % 2].dma_start(out=tl, in_=x_tile[t])
                if has_tail:
                    # overwrite the invalid trailing rows of each segment's
                    # last run with duplicates of the last valid rows
                    patch_ring = rings[t % 2] if t == n_tiles - 1 else nc.gpsimd
                    patch_ring.dma_start(
                        out=tl[RUNS - 1 :: RUNS, J - n_bad :, :],
                        in_=x_rows[t, :, count - n_bad : count],
                    )

                # fold 32 -> 1 along j: f32 -> bf16, bf16 tree, bf16 -> f32
                w = work_pool.tile([128, J // 2, D], bf16, tag="w")
                nc.vector.tensor_max(out=w, in0=tl[:, : J // 2], in1=tl[:, J // 2 :])
                k = J // 2
                while k > 2:
                    k //= 2
                    nc.vector.tensor_max(
                        out=w[:, :k], in0=w[:, :k], in1=w[:, k : 2 * k]
                    )
                wf = work_pool.tile([128, D], f32, tag="wf")
                nc.vector.tensor_max(out=wf, in0=w[:, 0], in1=w[:, 1])

                # transpose, reduce each segment's 16 columns
                pt = psum_pool.tile([128, SEGS_PER_TILE, RUNS], f32, tag="pt")
                nc.tensor.transpose(
                    pt.rearrange("p a b -> p (a b)"), wf, ident_f
                )
                nc.vector.reduce_max(
                    out=outbuf[:, g0 : g0 + SEGS_PER_TILE],
                    in_=pt,
                    axis=mybir.AxisListType.X,
                )

                # outbuf chunk c = segs [c*128, (c+1)*128) is complete once
                # its last tile has reduced; transpose it back to row-major
                # [seg, d] immediately instead of at the end
                if (t + 1) * SEGS_PER_TILE % 128 == 0:
                    c = ((t + 1) * SEGS_PER_TILE) // 128 - 1
                    ptc = psum_pool2.tile([128, 128], f32, tag="ot_ps")
                    nc.tensor.transpose(
                        ptc, outbuf[:, c * 128 : (c + 1) * 128], ident_f
                    )
                    nc.scalar.copy(ot[:, c], ptc)

            nc.sync.dma_start(
                out=out_t.rearrange("(c p) d -> p c d", p=128), in_=ot
            )
    nc.compile()
    return nc


def _numpy_fallback(x: np.ndarray, sizes: np.ndarray, w: int) -> np.ndarray:
    ends = np.cumsum(sizes)
    starts = ends - sizes
    out = np.full((sizes.shape[0], x.shape[1]), -np.inf, dtype=np.float32)
    for i in range(sizes.shape[0]):
        c = int(sizes[i]) - w + 1
        if c > 0:
            out[i] = x[int(starts[i]) : int(starts[i]) + c].max(axis=0)
    return out


def kernel(x, sizes, window_size) -> np.ndarray:
    x = np.ascontiguousarray(np.asarray(x, dtype=np.float32))
    sizes = np.asarray(sizes)
    w = int(np.asarray(window_size))
    n_seg = sizes.shape[0]
    count = SEG_LEN - w + 1

    uniform = (
        x.ndim == 2
        and x.shape[1] == D
        and bool((sizes == SEG_LEN).all())
        and x.shape[0] == n_seg * SEG_LEN
        and n_seg % (N_CORES * SEGS_PER_TILE) == 0
        and (n_seg // N_CORES) % 128 == 0
        and SEG_LEN - J < count <= SEG_LEN
        and count >= 2 * J
    )
    if not uniform:
        return _numpy_fallback(x, sizes, w)

    n_seg_core = n_seg // N_CORES
    key = (n_seg_core, count)
    if key not in _PROGRAM_CACHE:
        _PROGRAM_CACHE[key] = _build_program(n_seg_core, count)
    nc = _PROGRAM_CACHE[key]

    shards = np.split(x, N_CORES, axis=0)
    in_maps = [{"x": s} for s in shards]
    res = run_bass_kernel_spmd(nc, in_maps, core_ids=list(range(N_CORES)))
    return np.concatenate([r["out"] for r in res.results], axis=0)
